# revision 1
# baseline (speedup 1.0000x reference)
"""DFMConv2d Trainium2 kernel.

Reference computation (per sample b):
  pooled = mean_{h,w} x[b]                          [C=256]
  h      = relu(pooled @ w1.T + b1)                 [128]
  mix    = softmax((h @ w2.T + b2).reshape(256, 8)) [256, 8]
  y      = conv3x3_SAME(x[b], base_filters)         [8, 64, 64]
  out[b] = einsum('on,nhw->ohw', mix, y)            [256, 64, 64]

Strategy (8 NeuronCores, data-parallel over batch, 8 samples/core), all
heavy matmuls in float32r (~2e-4 rel err):

  conv:  y_tap[(t,n), hw] = sum_c filt[t,n,c] * x[c, hw] — all 9 taps in
         the stationary M dim (M=72), so x streams through the PE exactly
         twice; 16 matmuls/sample into a row-padded flat buffer
         ypad[72, 1+66*64+2] (rows -1 and 64 zeroed).
  shift: z[(t,n), hw] = y_tap shifted by (dy-1, dx-1) — one fully
         CONTIGUOUS SBUF->SBUF DMA per tap (offset dy*64+dx into ypad),
         then 6 tiny column-zero fixups for the dx!=1 wraparound cells.
  mix:   out[o, hw] = mixT72.T @ z with K=72; mixT72 = softmax(mix).T
         replicated 9x via 4 doubling partition-shift DMAs.
  DMA issue is split across rings: x loads on GPSIMD/SWDGE, out stores on
  the ACT HWDGE ring, z/mixT/params on the SP ring — avoids FIFO
  head-of-line blocking between pipeline stages.
"""
import sys

sys.path.insert(0, "/opt/trn_rl_repo")

import numpy as np
import ml_dtypes

import concourse.bass as bass
import concourse.bacc as bacc
import concourse.tile as tile
import concourse.mybir as mybir
from concourse.bass_utils import run_bass_kernel_spmd
from contextlib import ExitStack

F32 = mybir.dt.float32
F32R = mybir.dt.float32r
AFT = mybir.ActivationFunctionType
AXX = mybir.AxisListType.X
ALU = mybir.AluOpType

N_CORES = 8
BPC = 8            # samples per core
C = 256
CO = 256
H = W = 64
HW = H * W
NB = 8             # n_base
HID = 128
CCH = 2            # channel chunks of 128
NHC = 8            # h-chunks (8 output rows each)
NT = 9             # taps
M88 = 88           # taps grouped by dx at 32-aligned bases: rows 32*dx+8*dy..+8
YP_LEN = 1 + 66 * 64 + 2   # lead zero + 66 rows + tail slack (reads reach 4225)
TAP_ROW = {(dy, dx): 32 * dx + 8 * dy for dy in range(3) for dx in range(3)}

_BUILT = None


def _build():
    nc = bacc.Bacc("TRN2", target_bir_lowering=False)

    d_x = nc.dram_tensor("x", [BPC, C, HW], F32R, kind="ExternalInput")
    d_w1t = nc.dram_tensor("w1t", [C, HID], F32, kind="ExternalInput")
    d_b1 = nc.dram_tensor("b1", [HID, 1], F32, kind="ExternalInput")
    d_w2p = nc.dram_tensor("w2p", [HID, NB, CO], F32, kind="ExternalInput")
    d_b2t = nc.dram_tensor("b2t", [128, 2, NB], F32, kind="ExternalInput")
    d_ft = nc.dram_tensor("ft", [128, CCH, M88], F32R, kind="ExternalInput")
    d_id = nc.dram_tensor("ident", [128, 128], F32, kind="ExternalInput")
    d_z0 = nc.dram_tensor("zeros", [128, 66], F32R, kind="ExternalInput")
    d_out = nc.dram_tensor("out", [BPC, 2, 128, HW], F32, kind="ExternalOutput")

    with tile.TileContext(nc) as tc, ExitStack() as ctx:
        prm = ctx.enter_context(tc.tile_pool(name="prm", bufs=1))
        xp = ctx.enter_context(tc.tile_pool(name="xp", bufs=2))
        ypp = ctx.enter_context(tc.tile_pool(name="ypp", bufs=2))
        zp = ctx.enter_context(tc.tile_pool(name="zp", bufs=2))
        op = ctx.enter_context(tc.tile_pool(name="op", bufs=3))
        sm = ctx.enter_context(tc.tile_pool(name="sm", bufs=2))
        ps_c = ctx.enter_context(tc.tile_pool(name="ps_c", bufs=2, space="PSUM"))
        ps_m = ctx.enter_context(tc.tile_pool(name="ps_m", bufs=3, space="PSUM"))
        ps_s = ctx.enter_context(tc.tile_pool(name="ps_s", bufs=2, space="PSUM"))

        # ---- params (loaded once) ----
        w1t_sb = prm.tile([128, CCH, HID], F32, tag="w1t")
        nc.sync.dma_start(out=w1t_sb, in_=d_w1t[:, :].rearrange("(cc p) h -> p cc h", p=128))
        b1_sb = prm.tile([128, 1], F32, tag="b1")
        nc.sync.dma_start(out=b1_sb, in_=d_b1[:, :])
        w2p_sb = prm.tile([HID, NB, CO], F32, tag="w2p")
        nc.sync.dma_start(out=w2p_sb, in_=d_w2p[:, :, :])
        b2t_sb = prm.tile([128, 2, NB], F32, tag="b2t")
        nc.sync.dma_start(out=b2t_sb, in_=d_b2t[:, :, :])
        ft_sb = prm.tile([128, CCH, M88], F32R, tag="ft")
        nc.sync.dma_start(out=ft_sb, in_=d_ft[:, :, :])
        id_sb = prm.tile([128, 128], F32, tag="ident")
        nc.sync.dma_start(out=id_sb, in_=d_id[:, :])
        z0_sb = prm.tile([128, 66], F32R, tag="z0")
        nc.sync.dma_start(out=z0_sb, in_=d_z0[:, :])
        pooled_sb = prm.tile([128, CCH, BPC], F32, tag="pooled")
        h_sb = prm.tile([128, BPC], F32, tag="h")
        trash = prm.tile([128, HW], F32, tag="trash")

        for j in range(BPC):
            # ---- load (SWDGE ring) + pooling (split DVE / ACT-accum) ----
            xt = xp.tile([128, CCH, HW], F32R, tag="x")
            nc.gpsimd.dma_start(
                out=xt, in_=d_x[j, :, :].rearrange("(cc p) hw -> p cc hw", p=128))
            nc.vector.reduce_sum(
                pooled_sb[:, 0, j:j + 1], xt[:, 0, :].bitcast(F32), axis=AXX)
            nc.scalar.activation(out=trash, in_=xt[:, 1, :].bitcast(F32),
                                 func=AFT.Copy, accum_out=pooled_sb[:, 1, j:j + 1])

            # ---- attention MLP (fp32) ----
            ph = ps_s.tile([128, 1], F32, tag="sm")
            for cc in range(CCH):
                nc.tensor.matmul(ph, w1t_sb[:, cc, :], pooled_sb[:, cc, j:j + 1],
                                 start=(cc == 0), stop=(cc == 1))
            nc.scalar.activation(out=h_sb[:, j:j + 1], in_=ph, func=AFT.Relu,
                                 bias=b1_sb, scale=1.0)

            mixT_sb = sm.tile([M88, 2, 128], F32R, tag="mixT")
            for oc in range(2):
                pl = ps_s.tile([128, NB], F32, tag="sm")
                for n in range(NB):
                    nc.tensor.matmul(pl[:, n:n + 1],
                                     w2p_sb[:, n, oc * 128:(oc + 1) * 128],
                                     h_sb[:, j:j + 1], start=True, stop=True)
                lg_sb = sm.tile([128, NB], F32, tag="lg_sb")
                nc.vector.tensor_tensor(out=lg_sb, in0=pl, in1=b2t_sb[:, oc, :],
                                        op=ALU.add)
                ex_sb = sm.tile([128, NB], F32, tag="ex_sb")
                nc.scalar.activation(out=ex_sb, in_=lg_sb, func=AFT.Exp)
                sums = sm.tile([128, 1], F32, tag="sums")
                nc.vector.reduce_sum(sums, ex_sb, axis=AXX)
                rec = sm.tile([128, 1], F32, tag="rec")
                nc.vector.reciprocal(rec, sums)
                mix_sb = sm.tile([128, NB], F32, tag="mix_sb")
                nc.vector.tensor_scalar_mul(out=mix_sb, in0=ex_sb, scalar1=rec)
                ptr = ps_s.tile([NB, 128], F32, tag="sm")
                nc.tensor.transpose(ptr, mix_sb, id_sb)
                # DVE cast fp32 -> f32r counts as a rounding producer
                nc.vector.tensor_copy(mixT_sb[0:NB, oc, :], ptr)
            # replicate rows [0:8) nine times via doubling partition-shift DMAs
            nc.sync.dma_start(out=mixT_sb[8:16], in_=mixT_sb[0:8])
            nc.sync.dma_start(out=mixT_sb[16:32], in_=mixT_sb[0:16])
            nc.sync.dma_start(out=mixT_sb[32:64], in_=mixT_sb[0:32])
            nc.sync.dma_start(out=mixT_sb[64:88], in_=mixT_sb[0:24])

            # ---- conv into row-padded flat y_tap ----
            ypad = ypp.tile([M88, YP_LEN], F32R, tag="ypad")
            nc.vector.tensor_copy(ypad[:, 0:65].bitcast(F32),
                                  z0_sb[0:M88, 0:65].bitcast(F32))
            nc.vector.tensor_copy(ypad[:, 4161:4226].bitcast(F32),
                                  z0_sb[0:M88, 0:65].bitcast(F32))
            for hc in range(NHC):
                yps = ps_c.tile([128, 512], F32, tag="yps")
                for cc in range(CCH):
                    nc.tensor.matmul(yps[0:M88, :], ft_sb[:, cc, :],
                                     xt[:, cc, 512 * hc:512 * (hc + 1)],
                                     start=(cc == 0), stop=(cc == 1))
                nc.scalar.copy(
                    out=ypad[:, 65 + 512 * hc:65 + 512 * (hc + 1)].bitcast(F32),
                    in_=yps[0:M88, :])

            # ---- per-tap shifted windows into z (contiguous DMAs) ----
            zt = zp.tile([M88, HW], F32R, tag="z")
            ztv = zt.rearrange("p (h w) -> p h w", w=64)
            for dy in range(3):
                for dx in range(3):
                    r = TAP_ROW[(dy, dx)]
                    off = dy * 64 + dx
                    # dy=2 taps in dx groups 0,1 also copy the zeroed gap rows
                    # (ypad rows r+8..r+16 are zero via the zero filter cols),
                    # so z has no uninitialized rows under the K=88 contraction
                    nr = 16 if (dy == 2 and dx < 2) else NB
                    nc.sync.dma_start(out=zt[r:r + nr, :],
                                      in_=ypad[r:r + nr, off:off + HW])
            # zero the dx wraparound columns: col 0 for dx=0 (rows 0:24),
            # col 63 for dx=2 (rows 64:88)
            nc.vector.tensor_copy(
                ztv[0:24, :, 0:1].rearrange("p h w -> p (h w)"),
                z0_sb[0:24, 0:64].bitcast(F32))
            nc.vector.tensor_copy(
                ztv[64:88, :, 63:64].rearrange("p h w -> p (h w)"),
                z0_sb[64:88, 0:64].bitcast(F32))

            # ---- mix: out[o, hw] = mixT72.T @ z (K=72, f32r) ----
            for oc in range(2):
                ot = op.tile([128, HW], F32, tag="out")
                for hc in range(NHC):
                    om = ps_m.tile([128, 512], F32, tag="ops")
                    nc.tensor.matmul(om, mixT_sb[:, oc, :],
                                     zt[:, 512 * hc:512 * (hc + 1)],
                                     start=True, stop=True)
                    if hc % 2 == 0:
                        nc.vector.tensor_copy(ot[:, 512 * hc:512 * (hc + 1)], om)
                    else:
                        nc.scalar.copy(out=ot[:, 512 * hc:512 * (hc + 1)], in_=om)
                nc.scalar.dma_start(out=d_out[j, oc, :, :], in_=ot)

    nc.compile()
    return nc


def _prep_inputs(x, w1, b1, w2, b2, base_filters):
    """Host-side input layout prep. Returns per-core in_maps."""
    B = x.shape[0]
    xs = np.ascontiguousarray(x.reshape(B, C, HW)).astype(np.float32)
    w1t = np.ascontiguousarray(w1.T).astype(np.float32) / float(HW)
    b1c = np.ascontiguousarray(b1.reshape(HID, 1)).astype(np.float32)
    w2p = np.ascontiguousarray(w2.reshape(CO, NB, HID).transpose(2, 1, 0)).astype(np.float32)
    b2t = np.ascontiguousarray(b2.reshape(2, 128, NB).transpose(1, 0, 2)).astype(np.float32)
    filt = base_filters.reshape(NB, CCH, 128, 3, 3)  # [n, cc, cp, dy, dx]
    # ft[c_part, cc, 32*dx + 8*dy + n] = filt[n, cc, c_part, dy, dx]; gaps zero
    ft = np.zeros((128, CCH, M88), dtype=np.float32)
    for dy in range(3):
        for dx in range(3):
            r = 32 * dx + 8 * dy
            ft[:, :, r:r + NB] = filt[:, :, :, dy, dx].transpose(2, 1, 0)
    ident = np.eye(128, dtype=np.float32)
    zeros = np.zeros((128, 66), dtype=np.float32)

    in_maps = []
    for core in range(N_CORES):
        in_maps.append({
            "x": np.ascontiguousarray(xs[core * BPC:(core + 1) * BPC]),
            "w1t": w1t, "b1": b1c, "w2p": w2p, "b2t": b2t,
            "ft": ft, "ident": ident, "zeros": zeros,
        })
    return in_maps


def kernel(x, w1, b1, w2, b2, base_filters):
    global _BUILT
    if _BUILT is None:
        _BUILT = _build()
    nc = _BUILT
    in_maps = _prep_inputs(np.asarray(x, dtype=np.float32),
                           np.asarray(w1, dtype=np.float32),
                           np.asarray(b1, dtype=np.float32),
                           np.asarray(w2, dtype=np.float32),
                           np.asarray(b2, dtype=np.float32),
                           np.asarray(base_filters, dtype=np.float32))
    res = run_bass_kernel_spmd(nc, in_maps, core_ids=list(range(N_CORES)))
    outs = []
    for core in range(N_CORES):
        o = res.results[core]["out"]            # [BPC, 2, 128, HW]
        outs.append(o.reshape(BPC, CO, H, W))
    return np.concatenate(outs, axis=0).astype(np.float32)



# revision 13
# speedup vs baseline: 1.1020x; 1.1020x over previous
"""DFMConv2d Trainium2 kernel (v2: dy-accumulated conv, bf16 I/O).

Reference computation (per sample b):
  pooled = mean_{h,w} x[b]                          [C=256]
  h      = relu(pooled @ w1.T + b1)                 [128]
  mix    = softmax((h @ w2.T + b2).reshape(256, 8)) [256, 8]
  y      = conv3x3_SAME(x[b], base_filters)         [8, 64, 64]
  out[b] = einsum('on,nhw->ohw', mix, y)            [256, 64, 64]

Strategy (8 cores, data-parallel over batch, 8 samples/core), bf16:

  conv:  stationary M=24 holds (dx, n); the three dy taps ACCUMULATE into
         one PSUM [24, 512] via +/-64-column offsets on the moving x
         operand (row shifts are free; hc-edge blocks use partial column
         ranges so x needs no padding and stays 16KB-contiguous).
  shift: only dx = +/-1 column remains: 3 engine copies (DVE + GpSimd)
         z24 -> zs, plus tiny wrap-column memsets. No SBUF->SBUF DMAs.
  mix:   out[o, hw] = mixT24.T @ zs with K=24; output channel o = 2p+oc
         folded into the w2 column permutation so stores are contiguous.
  MLP:   batched 4 samples at a time; softmax normalization deferred to
         the final PSUM->SBUF copy via per-partition scale (1/sum).
  DMA:   x loads on GpSimd SWDGE ring (sample 0 on sync HWDGE), out
         stores on scalar HWDGE ring; everything bf16 => 2MB/sample each
         way in 16KB-contiguous runs.
"""
import sys

sys.path.insert(0, "/opt/trn_rl_repo")

import numpy as np
import ml_dtypes

import concourse.bass as bass
import concourse.bacc as bacc
import concourse.tile as tile
import concourse.mybir as mybir
from concourse.bass_utils import run_bass_kernel_spmd
from contextlib import ExitStack

F32 = mybir.dt.float32
BF16 = mybir.dt.bfloat16
AFT = mybir.ActivationFunctionType
AXX = mybir.AxisListType.X
ALU = mybir.AluOpType

N_CORES = 8
BPC = 8            # samples per core
C = 256
CO = 256
H = W = 64
HW = H * W
NB = 8             # n_base
HID = 128
M72 = 72           # stationary rows: 32*dx + n (32-aligned dx groups)
ZLEN = 1 + HW + 1  # z24 row: lead zero col + 4096 + tail zero col

_BUILT = None


def _build():
    nc = bacc.Bacc("TRN2", target_bir_lowering=False)

    d_x = nc.dram_tensor("x", [128, BPC, 2, HW], BF16, kind="ExternalInput")
    d_ft = nc.dram_tensor("ft", [128, 2, 3, M72], BF16, kind="ExternalInput")
    d_w1t = nc.dram_tensor("w1t", [128, 2, HID], F32, kind="ExternalInput")
    d_b1 = nc.dram_tensor("b1", [HID, 1], F32, kind="ExternalInput")
    d_w2p = nc.dram_tensor("w2p", [HID, 2, NB, 128], F32, kind="ExternalInput")
    d_b2r = nc.dram_tensor("b2r", [128, 2, NB, BPC], F32, kind="ExternalInput")
    d_id = nc.dram_tensor("ident", [128, 128], F32, kind="ExternalInput")
    d_out = nc.dram_tensor("out", [128, BPC, 2, HW], BF16, kind="ExternalOutput")

    with tile.TileContext(nc) as tc, ExitStack() as ctx:
        prm = ctx.enter_context(tc.tile_pool(name="prm", bufs=1))
        xp = ctx.enter_context(tc.tile_pool(name="xp", bufs=3))
        z24p = ctx.enter_context(tc.tile_pool(name="z24p", bufs=2))
        zsp = ctx.enter_context(tc.tile_pool(name="zsp", bufs=5))
        op = ctx.enter_context(tc.tile_pool(name="op", bufs=3))
        mtp = ctx.enter_context(tc.tile_pool(name="mtp", bufs=2))
        sm = ctx.enter_context(tc.tile_pool(name="sm", bufs=2))
        ps_c = ctx.enter_context(tc.tile_pool(name="ps_c", bufs=2, space="PSUM"))
        ps_m = ctx.enter_context(tc.tile_pool(name="ps_m", bufs=3, space="PSUM"))
        ps_s = ctx.enter_context(tc.tile_pool(name="ps_s", bufs=1, space="PSUM"))

        # ---- params (loaded once, sync HW ring) ----
        ft_sb = prm.tile([128, 2, 3, M72], BF16, tag="ft")
        nc.sync.dma_start(out=ft_sb, in_=d_ft[:, :, :, :])
        w1t_sb = prm.tile([128, 2, HID], F32, tag="w1t")
        nc.sync.dma_start(out=w1t_sb, in_=d_w1t[:, :, :])
        b1_sb = prm.tile([HID, 1], F32, tag="b1")
        nc.sync.dma_start(out=b1_sb, in_=d_b1[:, :])
        w2p_sb = prm.tile([HID, 2, NB, 128], F32, tag="w2p")
        nc.sync.dma_start(out=w2p_sb, in_=d_w2p[:, :, :, :])
        b2r_sb = prm.tile([128, 2, NB, BPC], F32, tag="b2r")
        nc.sync.dma_start(out=b2r_sb, in_=d_b2r[:, :, :, :])
        id_sb = prm.tile([128, 128], F32, tag="ident")
        nc.sync.dma_start(out=id_sb, in_=d_id[:, :])

        pooled01 = prm.tile([128, 2, BPC], F32, tag="pooled01")
        mr_sb = prm.tile([128, M72], F32, tag="mr")
        nc.vector.memset(mr_sb, 0)   # gap columns stay zero forever
        h_sb = prm.tile([HID, BPC], F32, tag="h")

        xts = [None] * BPC
        zss = [None] * BPC
        mixTs = [None] * 2   # per batch of 4
        recs = [None] * 2

        def mlp_batch(b):
            j0 = 4 * b
            ph = ps_s.tile([HID, 4], F32, tag="ph")
            for half in range(2):
                nc.tensor.matmul(ph, w1t_sb[:, half, :],
                                 pooled01[:, half, j0:j0 + 4],
                                 start=(half == 0), stop=(half == 1))
            nc.scalar.activation(out=h_sb[:, j0:j0 + 4], in_=ph, func=AFT.Relu,
                                 bias=b1_sb, scale=1.0)
            pl = ps_s.tile([128, 2, NB, 4], F32, tag="pl")
            for oc in range(2):
                for n in range(NB):
                    nc.tensor.matmul(pl[:, oc, n, :], w2p_sb[:, oc, n, :],
                                     h_sb[:, j0:j0 + 4], start=True, stop=True)
            lg = sm.tile([128, 2, NB, 4], F32, tag="lg")
            nc.vector.tensor_tensor(out=lg, in0=pl, in1=b2r_sb[:, :, :, 0:4],
                                    op=ALU.add)
            ex = sm.tile([128, 2, NB, 4], F32, tag="ex")
            nc.scalar.activation(out=ex, in_=lg, func=AFT.Exp)
            # transpose (n, j) -> (j, n) so the n-sum is innermost
            exT = sm.tile([128, 2, 4, NB], F32, tag="exT")
            nc.vector.tensor_copy(exT, ex.rearrange("p a n j -> p a j n"))
            sums = sm.tile([128, 2, 4], F32, tag="sums")
            nc.vector.reduce_sum(sums, exT, axis=AXX)
            rec_b = mtp.tile([128, 2, 4], F32, tag="rec")
            nc.vector.reciprocal(rec_b, sums)
            recs[b] = rec_b
            mixT_b = mtp.tile([M72, 2, 4, 128], BF16, tag="mixT")
            mixTs[b] = mixT_b
            for j2 in range(4):
                for oc in range(2):
                    for dx in range(3):
                        nc.vector.tensor_copy(mr_sb[:, 32 * dx:32 * dx + 8],
                                              exT[:, oc, j2, :])
                    ptr = ps_s.tile([M72, 128], F32, tag="ptr")
                    nc.tensor.transpose(ptr, mr_sb, id_sb)
                    nc.vector.tensor_copy(mixT_b[:, oc, j2, :], ptr)

        for j in range(BPC):
            # ---- load x (SWDGE ring; first sample on sync HW ring) ----
            xt = xp.tile([128, 2, HW], BF16, tag="x")
            xts[j] = xt
            ring = nc.sync if j == 0 else nc.gpsimd
            ring.dma_start(out=xt, in_=d_x[:, j, :, :])

            # ---- pooling (one DVE reduce over both halves) ----
            nc.vector.reduce_sum(pooled01[:, :, j:j + 1], xt, axis=AXX)

            # ---- conv: accumulate 3 dy taps x 2 channel halves in PSUM ----
            z24 = z24p.tile([M72, ZLEN], BF16, tag="z24")
            if j < 2:
                nc.vector.memset(z24[:, 0:1], 0)
                nc.vector.memset(z24[:, ZLEN - 1:ZLEN], 0)
            for hc in range(8):
                yps = ps_c.tile([M72, 512], F32, tag="yps")
                dys = (1, 2, 0) if hc == 7 else (1, 0, 2)
                k = 0
                for dy in dys:
                    lo = 512 * hc + 64 * (dy - 1)
                    hi = lo + 512
                    clo, chi = max(lo, 0), min(hi, HW)
                    for half in range(2):
                        nc.tensor.matmul(
                            yps[:, clo - lo:512 - (hi - chi)],
                            ft_sb[:, half, dy, :],
                            xt[:, half, clo:chi],
                            start=(k == 0), stop=(k == 5))
                        k += 1
                nc.scalar.copy(
                    out=z24[:, 1 + 512 * hc:1 + 512 * (hc + 1)], in_=yps)

            # ---- dx shift: z24 -> zs on DVE/GpSimd, then wrap fixups ----
            zs = zsp.tile([M72, HW], BF16, tag="zs")
            zss[j] = zs
            # copies span the zero gap rows too so every zs row is initialized
            nc.gpsimd.tensor_copy(zs[0:32, :], z24[0:32, 0:HW])
            nc.vector.tensor_copy(zs[32:64, :], z24[32:64, 1:1 + HW])
            nc.gpsimd.tensor_copy(zs[64:72, :], z24[64:72, 2:2 + HW])
            zv = zs.rearrange("p (h w) -> p h w", w=64)
            nc.vector.memset(zv[0:8, :, 0:1], 0)
            nc.vector.memset(zv[64:72, :, 63:64], 0)

            # ---- mix deferred: emitted per 4-sample batch after its MLP ----
            if j == 3 or j == 7:
                b = j // 4
                mlp_batch(b)
                mixT_b = mixTs[b]
                rec_b = recs[b]
                for j2 in range(4):
                    jj = 4 * b + j2
                    zsj = zss[jj]
                    ot = op.tile([128, 2, HW], BF16, tag="out")
                    for oc in range(2):
                        for hc in range(8):
                            om = ps_m.tile([128, 512], F32, tag="om")
                            nc.tensor.matmul(om, mixT_b[:, oc, j2, :],
                                             zsj[:, 512 * hc:512 * (hc + 1)],
                                             start=True, stop=True)
                            dst = ot[:, oc, 512 * hc:512 * (hc + 1)]
                            if hc % 2 == 0:
                                nc.vector.tensor_scalar_mul(
                                    out=dst, in0=om,
                                    scalar1=rec_b[:, oc, j2:j2 + 1])
                            else:
                                nc.scalar.activation(
                                    out=dst, in_=om, func=AFT.Copy,
                                    scale=rec_b[:, oc, j2:j2 + 1])
                    nc.scalar.dma_start(out=d_out[:, jj, :, :], in_=ot)

    nc.compile()
    return nc


def _prep_inputs(x, w1, b1, w2, b2, base_filters):
    """Host-side input layout prep. Returns per-core in_maps."""
    B = x.shape[0]
    # x[core*8+j, 2p+half, hw] -> [128(p), 8(j), 2(half), hw] bf16
    xs = x.reshape(B, 128, 2, HW)
    # w1t[p, half, hid] = w1[hid, 2p+half] / HW  (mean folded in)
    w1t = np.ascontiguousarray(
        w1.reshape(HID, 128, 2).transpose(1, 2, 0)).astype(np.float32) / float(HW)
    b1c = np.ascontiguousarray(b1.reshape(HID, 1)).astype(np.float32)
    # w2 row index = o*NB + n with o = 2p+oc -> w2p[hid, oc, n, p]
    w2r = w2.reshape(128, 2, NB, HID)          # [p, oc, n, hid]
    w2p = np.ascontiguousarray(w2r.transpose(3, 1, 2, 0)).astype(np.float32)
    b2r = np.broadcast_to(
        b2.reshape(128, 2, NB, 1), (128, 2, NB, BPC))
    b2r = np.ascontiguousarray(b2r).astype(np.float32)
    # ft[p, half, dy, 32dx+n] = filt[n, 2p+half, dy, dx]; gaps zero
    filt = base_filters.reshape(NB, 128, 2, 3, 3)
    fpv = filt.transpose(1, 2, 3, 4, 0)                 # [p, half, dy, dx, n]
    ft = np.zeros((128, 2, 3, M72), dtype=np.float32)
    for dx in range(3):
        ft[:, :, :, 32 * dx:32 * dx + NB] = fpv[:, :, :, dx, :]
    ft = ft.astype(ml_dtypes.bfloat16)
    ident = np.eye(128, dtype=np.float32)

    in_maps = []
    for core in range(N_CORES):
        xc = np.ascontiguousarray(
            xs[core * BPC:(core + 1) * BPC].transpose(1, 0, 2, 3)).astype(
                ml_dtypes.bfloat16)
        in_maps.append({
            "x": xc, "ft": ft, "w1t": w1t, "b1": b1c, "w2p": w2p,
            "b2r": b2r, "ident": ident,
        })
    return in_maps


def kernel(x, w1, b1, w2, b2, base_filters):
    global _BUILT
    if _BUILT is None:
        _BUILT = _build()
    nc = _BUILT
    in_maps = _prep_inputs(np.asarray(x, dtype=np.float32),
                           np.asarray(w1, dtype=np.float32),
                           np.asarray(b1, dtype=np.float32),
                           np.asarray(w2, dtype=np.float32),
                           np.asarray(b2, dtype=np.float32),
                           np.asarray(base_filters, dtype=np.float32))
    res = run_bass_kernel_spmd(nc, in_maps, core_ids=list(range(N_CORES)))
    outs = []
    for core in range(N_CORES):
        o = res.results[core]["out"]            # [128, BPC, 2, HW] bf16
        o = np.asarray(o).astype(np.float32).transpose(1, 0, 2, 3)
        outs.append(o.reshape(BPC, CO, H, W))
    return np.concatenate(outs, axis=0).astype(np.float32)


# revision 15
# speedup vs baseline: 1.7158x; 1.5570x over previous
"""DFMConv2d Trainium2 kernel (v2: dy-accumulated conv, bf16 I/O).

Reference computation (per sample b):
  pooled = mean_{h,w} x[b]                          [C=256]
  h      = relu(pooled @ w1.T + b1)                 [128]
  mix    = softmax((h @ w2.T + b2).reshape(256, 8)) [256, 8]
  y      = conv3x3_SAME(x[b], base_filters)         [8, 64, 64]
  out[b] = einsum('on,nhw->ohw', mix, y)            [256, 64, 64]

Strategy (8 cores, data-parallel over batch, 8 samples/core), bf16:

  conv:  stationary M=24 holds (dx, n); the three dy taps ACCUMULATE into
         one PSUM [24, 512] via +/-64-column offsets on the moving x
         operand (row shifts are free; hc-edge blocks use partial column
         ranges so x needs no padding and stays 16KB-contiguous).
  shift: only dx = +/-1 column remains: 3 engine copies (DVE + GpSimd)
         z24 -> zs, plus tiny wrap-column memsets. No SBUF->SBUF DMAs.
  mix:   out[o, hw] = mixT24.T @ zs with K=24; output channel o = 2p+oc
         folded into the w2 column permutation so stores are contiguous.
  MLP:   batched 4 samples at a time; softmax normalization deferred to
         the final PSUM->SBUF copy via per-partition scale (1/sum).
  DMA:   x loads on GpSimd SWDGE ring (sample 0 on sync HWDGE), out
         stores on scalar HWDGE ring; everything bf16 => 2MB/sample each
         way in 16KB-contiguous runs.
"""
import sys

sys.path.insert(0, "/opt/trn_rl_repo")

import numpy as np
import ml_dtypes

import concourse.bass as bass
import concourse.bacc as bacc
import concourse.tile as tile
import concourse.mybir as mybir
from concourse.bass_utils import run_bass_kernel_spmd
from contextlib import ExitStack

F32 = mybir.dt.float32
BF16 = mybir.dt.bfloat16
AFT = mybir.ActivationFunctionType
AXX = mybir.AxisListType.X
ALU = mybir.AluOpType

N_CORES = 8
BPC = 8            # samples per core
C = 256
CO = 256
H = W = 64
HW = H * W
NB = 8             # n_base
HID = 128
M72 = 72           # stationary rows: 32*dx + n (32-aligned dx groups)
ZLEN = 1 + HW + 1  # z24 row: lead zero col + 4096 + tail zero col

_BUILT = None


def _build():
    nc = bacc.Bacc("TRN2", target_bir_lowering=False)

    d_x = nc.dram_tensor("x", [128, BPC, 2, HW], BF16, kind="ExternalInput")
    d_ft = nc.dram_tensor("ft", [128, 2, 3, M72], BF16, kind="ExternalInput")
    d_w1tb = nc.dram_tensor("w1tb", [128, 2, HID], BF16, kind="ExternalInput")
    d_b1 = nc.dram_tensor("b1", [HID, 1], F32, kind="ExternalInput")
    d_w2p = nc.dram_tensor("w2p", [HID, 2, NB, 128], F32, kind="ExternalInput")
    d_b2r = nc.dram_tensor("b2r", [128, 2, NB, BPC], F32, kind="ExternalInput")
    d_id = nc.dram_tensor("ident", [128, 128], F32, kind="ExternalInput")
    d_out = nc.dram_tensor("out", [128, BPC, 2, HW], BF16, kind="ExternalOutput")

    with tile.TileContext(nc) as tc, ExitStack() as ctx:
        prm = ctx.enter_context(tc.tile_pool(name="prm", bufs=1))
        xp = ctx.enter_context(tc.tile_pool(name="xp", bufs=3))
        z24p = ctx.enter_context(tc.tile_pool(name="z24p", bufs=2))
        zsp = ctx.enter_context(tc.tile_pool(name="zsp", bufs=5))
        op = ctx.enter_context(tc.tile_pool(name="op", bufs=3))
        mtp = ctx.enter_context(tc.tile_pool(name="mtp", bufs=2))
        sm = ctx.enter_context(tc.tile_pool(name="sm", bufs=2))
        ps_c = ctx.enter_context(tc.tile_pool(name="ps_c", bufs=2, space="PSUM"))
        ps_m = ctx.enter_context(tc.tile_pool(name="ps_m", bufs=3, space="PSUM"))
        ps_s = ctx.enter_context(tc.tile_pool(name="ps_s", bufs=1, space="PSUM"))
        ps_u = ctx.enter_context(tc.tile_pool(name="ps_u", bufs=1, space="PSUM"))

        # ---- params (loaded once, sync HW ring) ----
        ft_sb = prm.tile([128, 2, 3, M72], BF16, tag="ft")
        nc.sync.dma_start(out=ft_sb, in_=d_ft[:, :, :, :])
        w1tb_sb = prm.tile([128, 2, HID], BF16, tag="w1tb")
        nc.sync.dma_start(out=w1tb_sb, in_=d_w1tb[:, :, :])
        b1_sb = prm.tile([HID, 1], F32, tag="b1")
        nc.sync.dma_start(out=b1_sb, in_=d_b1[:, :])
        w2p_sb = prm.tile([HID, 2, NB, 128], F32, tag="w2p")
        nc.sync.dma_start(out=w2p_sb, in_=d_w2p[:, :, :, :])
        b2r_sb = prm.tile([128, 2, NB, BPC], F32, tag="b2r")
        nc.sync.dma_start(out=b2r_sb, in_=d_b2r[:, :, :, :])
        id_sb = prm.tile([128, 128], F32, tag="ident")
        nc.sync.dma_start(out=id_sb, in_=d_id[:, :])

        hraw_sb = prm.tile([HID, BPC], F32, tag="hraw")
        mr_sb = prm.tile([128, M72], F32, tag="mr")
        nc.vector.memset(mr_sb, 0)   # gap columns stay zero forever
        h_sb = prm.tile([HID, BPC], F32, tag="h")

        xts = [None] * BPC
        zss = [None] * BPC
        mixTs = [None] * 2   # per batch of 4
        recs = [None] * 2

        def mlp_batch(b):
            j0 = 4 * b
            nc.scalar.activation(out=h_sb[:, j0:j0 + 4],
                                 in_=hraw_sb[:, j0:j0 + 4], func=AFT.Relu,
                                 bias=b1_sb, scale=1.0)
            pl = ps_s.tile([128, 2, NB, 4], F32, tag="pl")
            for oc in range(2):
                for n in range(NB):
                    nc.tensor.matmul(pl[:, oc, n, :], w2p_sb[:, oc, n, :],
                                     h_sb[:, j0:j0 + 4], start=True, stop=True)
            lg = sm.tile([128, 2, NB, 4], F32, tag="lg")
            nc.vector.tensor_tensor(out=lg, in0=pl, in1=b2r_sb[:, :, :, 0:4],
                                    op=ALU.add)
            ex = sm.tile([128, 2, NB, 4], F32, tag="ex")
            nc.scalar.activation(out=ex, in_=lg, func=AFT.Exp)
            # transpose (n, j) -> (j, n) so the n-sum is innermost
            exT = sm.tile([128, 2, 4, NB], F32, tag="exT")
            nc.vector.tensor_copy(exT, ex.rearrange("p a n j -> p a j n"))
            sums = sm.tile([128, 2, 4], F32, tag="sums")
            nc.vector.reduce_sum(sums, exT, axis=AXX)
            rec_b = mtp.tile([128, 2, 4], F32, tag="rec")
            nc.vector.reciprocal(rec_b, sums)
            recs[b] = rec_b
            mixT_b = mtp.tile([M72, 2, 4, 128], BF16, tag="mixT")
            mixTs[b] = mixT_b
            for j2 in range(4):
                for oc in range(2):
                    for dx in range(3):
                        nc.vector.tensor_copy(mr_sb[:, 32 * dx:32 * dx + 8],
                                              exT[:, oc, j2, :])
                    ptr = ps_s.tile([M72, 128], F32, tag="ptr")
                    nc.tensor.transpose(ptr, mr_sb, id_sb)
                    nc.vector.tensor_copy(mixT_b[:, oc, j2, :], ptr)

        for j in range(BPC):
            # ---- load x (SWDGE ring; first sample on sync HW ring) ----
            xt = xp.tile([128, 2, HW], BF16, tag="x")
            xts[j] = xt
            nc.gpsimd.dma_start(out=xt, in_=d_x[:, j, :, :])

            # ---- conv: accumulate 3 dy taps x 2 channel halves in PSUM ----
            z24 = z24p.tile([M72, ZLEN], BF16, tag="z24")
            if j < 2:
                nc.vector.memset(z24[:, 0:1], 0)
                nc.vector.memset(z24[:, ZLEN - 1:ZLEN], 0)
            ups = ps_u.tile([HID, 512], F32, tag="u")
            for hc in range(8):
                yps = ps_c.tile([M72, 512], F32, tag="yps")
                dys = (1, 2, 0) if hc == 7 else (1, 0, 2)
                k = 0
                for dy in dys:
                    lo = 512 * hc + 64 * (dy - 1)
                    hi = lo + 512
                    clo, chi = max(lo, 0), min(hi, HW)
                    for half in range(2):
                        nc.tensor.matmul(
                            yps[:, clo - lo:512 - (hi - chi)],
                            ft_sb[:, half, dy, :],
                            xt[:, half, clo:chi],
                            start=(k == 0), stop=(k == 5))
                        k += 1
                for half in range(2):
                    nc.tensor.matmul(
                        ups, w1tb_sb[:, half, :],
                        xt[:, half, 512 * hc:512 * (hc + 1)],
                        start=(hc == 0 and half == 0),
                        stop=(hc == 7 and half == 1))
                nc.scalar.copy(
                    out=z24[:, 1 + 512 * hc:1 + 512 * (hc + 1)], in_=yps)
            nc.vector.reduce_sum(hraw_sb[:, j:j + 1], ups, axis=AXX)

            # ---- dx shift: z24 -> zs on DVE/GpSimd, then wrap fixups ----
            zs = zsp.tile([M72, HW], BF16, tag="zs")
            zss[j] = zs
            if j < 5:
                # first use of each pool buffer: zero it so gap rows and
                # wrap columns are zero forever after
                nc.vector.memset(zs[0:64, :], 0)
                nc.vector.memset(zs[64:72, :], 0)
            nc.sync.dma_start(out=zs[0:8, :], in_=z24[0:8, 0:HW])
            nc.sync.dma_start(out=zs[32:40, :], in_=z24[32:40, 1:1 + HW])
            nc.sync.dma_start(out=zs[64:72, :], in_=z24[64:72, 2:2 + HW])
            zv = zs.rearrange("p (h w) -> p h w", w=64)
            nc.vector.memset(zv[0:8, :, 0:1], 0)
            nc.vector.memset(zv[64:72, :, 63:64], 0)

            # ---- mix deferred: emitted per 4-sample batch after its MLP ----
            if j == 3 or j == 7:
                b = j // 4
                mlp_batch(b)
                mixT_b = mixTs[b]
                rec_b = recs[b]
                for j2 in range(4):
                    jj = 4 * b + j2
                    zsj = zss[jj]
                    ot = op.tile([128, 2, HW], BF16, tag="out")
                    for oc in range(2):
                        for hc in range(8):
                            om = ps_m.tile([128, 512], F32, tag="om")
                            nc.tensor.matmul(om, mixT_b[:, oc, j2, :],
                                             zsj[:, 512 * hc:512 * (hc + 1)],
                                             start=True, stop=True)
                            dst = ot[:, oc, 512 * hc:512 * (hc + 1)]
                            if hc % 2 == 0:
                                nc.vector.tensor_scalar_mul(
                                    out=dst, in0=om,
                                    scalar1=rec_b[:, oc, j2:j2 + 1])
                            else:
                                nc.scalar.activation(
                                    out=dst, in_=om, func=AFT.Copy,
                                    scale=rec_b[:, oc, j2:j2 + 1])
                    oring = nc.gpsimd if jj == 0 else nc.scalar
                    oring.dma_start(out=d_out[:, jj, :, :], in_=ot)

    nc.compile()
    return nc


def _prep_inputs(x, w1, b1, w2, b2, base_filters):
    """Host-side input layout prep. Returns per-core in_maps."""
    B = x.shape[0]
    # x[core*8+j, 2p+half, hw] -> [128(p), 8(j), 2(half), hw] bf16
    xs = x.reshape(B, 128, 2, HW)
    # w1t[p, half, hid] = w1[hid, 2p+half] / HW  (mean folded in)
    w1t = (np.ascontiguousarray(
        w1.reshape(HID, 128, 2).transpose(1, 2, 0)).astype(np.float32)
        / float(HW)).astype(ml_dtypes.bfloat16)
    b1c = np.ascontiguousarray(b1.reshape(HID, 1)).astype(np.float32)
    # w2 row index = o*NB + n with o = 2p+oc -> w2p[hid, oc, n, p]
    w2r = w2.reshape(128, 2, NB, HID)          # [p, oc, n, hid]
    w2p = np.ascontiguousarray(w2r.transpose(3, 1, 2, 0)).astype(np.float32)
    b2r = np.broadcast_to(
        b2.reshape(128, 2, NB, 1), (128, 2, NB, BPC))
    b2r = np.ascontiguousarray(b2r).astype(np.float32)
    # ft[p, half, dy, 32dx+n] = filt[n, 2p+half, dy, dx]; gaps zero
    filt = base_filters.reshape(NB, 128, 2, 3, 3)
    fpv = filt.transpose(1, 2, 3, 4, 0)                 # [p, half, dy, dx, n]
    ft = np.zeros((128, 2, 3, M72), dtype=np.float32)
    for dx in range(3):
        ft[:, :, :, 32 * dx:32 * dx + NB] = fpv[:, :, :, dx, :]
    ft = ft.astype(ml_dtypes.bfloat16)
    ident = np.eye(128, dtype=np.float32)

    in_maps = []
    for core in range(N_CORES):
        xc = np.ascontiguousarray(
            xs[core * BPC:(core + 1) * BPC].transpose(1, 0, 2, 3)).astype(
                ml_dtypes.bfloat16)
        in_maps.append({
            "x": xc, "ft": ft, "w1tb": w1t, "b1": b1c, "w2p": w2p,
            "b2r": b2r, "ident": ident,
        })
    return in_maps


def kernel(x, w1, b1, w2, b2, base_filters):
    global _BUILT
    if _BUILT is None:
        _BUILT = _build()
    nc = _BUILT
    in_maps = _prep_inputs(np.asarray(x, dtype=np.float32),
                           np.asarray(w1, dtype=np.float32),
                           np.asarray(b1, dtype=np.float32),
                           np.asarray(w2, dtype=np.float32),
                           np.asarray(b2, dtype=np.float32),
                           np.asarray(base_filters, dtype=np.float32))
    res = run_bass_kernel_spmd(nc, in_maps, core_ids=list(range(N_CORES)))
    outs = []
    for core in range(N_CORES):
        o = res.results[core]["out"]            # [128, BPC, 2, HW] bf16
        o = np.asarray(o).astype(np.float32).transpose(1, 0, 2, 3)
        outs.append(o.reshape(BPC, CO, H, W))
    return np.concatenate(outs, axis=0).astype(np.float32)


# revision 16
# speedup vs baseline: 1.9284x; 1.1239x over previous
"""DFMConv2d Trainium2 kernel (v2: dy-accumulated conv, bf16 I/O).

Reference computation (per sample b):
  pooled = mean_{h,w} x[b]                          [C=256]
  h      = relu(pooled @ w1.T + b1)                 [128]
  mix    = softmax((h @ w2.T + b2).reshape(256, 8)) [256, 8]
  y      = conv3x3_SAME(x[b], base_filters)         [8, 64, 64]
  out[b] = einsum('on,nhw->ohw', mix, y)            [256, 64, 64]

Strategy (8 cores, data-parallel over batch, 8 samples/core), bf16:

  conv:  stationary M=24 holds (dx, n); the three dy taps ACCUMULATE into
         one PSUM [24, 512] via +/-64-column offsets on the moving x
         operand (row shifts are free; hc-edge blocks use partial column
         ranges so x needs no padding and stays 16KB-contiguous).
  shift: only dx = +/-1 column remains: 3 engine copies (DVE + GpSimd)
         z24 -> zs, plus tiny wrap-column memsets. No SBUF->SBUF DMAs.
  mix:   out[o, hw] = mixT24.T @ zs with K=24; output channel o = 2p+oc
         folded into the w2 column permutation so stores are contiguous.
  MLP:   batched 4 samples at a time; softmax normalization deferred to
         the final PSUM->SBUF copy via per-partition scale (1/sum).
  DMA:   x loads on GpSimd SWDGE ring (sample 0 on sync HWDGE), out
         stores on scalar HWDGE ring; everything bf16 => 2MB/sample each
         way in 16KB-contiguous runs.
"""
import sys

sys.path.insert(0, "/opt/trn_rl_repo")

import numpy as np
import ml_dtypes

import concourse.bass as bass
import concourse.bacc as bacc
import concourse.tile as tile
import concourse.mybir as mybir
from concourse.bass_utils import run_bass_kernel_spmd
from contextlib import ExitStack

F32 = mybir.dt.float32
BF16 = mybir.dt.bfloat16
AFT = mybir.ActivationFunctionType
AXX = mybir.AxisListType.X
ALU = mybir.AluOpType

N_CORES = 8
BPC = 8            # samples per core
C = 256
CO = 256
H = W = 64
HW = H * W
NB = 8             # n_base
HID = 128
M72 = 72           # stationary rows: 32*dx + n (32-aligned dx groups)
ZLEN = 1 + HW + 1  # z24 row: lead zero col + 4096 + tail zero col

_BUILT = None


def _build():
    nc = bacc.Bacc("TRN2", target_bir_lowering=False)

    d_x = nc.dram_tensor("x", [128, BPC, 2, HW], BF16, kind="ExternalInput")
    d_ft = nc.dram_tensor("ft", [128, 2, 3, 128], BF16, kind="ExternalInput")
    d_w1tb = nc.dram_tensor("w1tb", [128, 2, HID], BF16, kind="ExternalInput")
    d_b1 = nc.dram_tensor("b1", [HID, 1], F32, kind="ExternalInput")
    d_w2p = nc.dram_tensor("w2p", [HID, 2, NB, 128], F32, kind="ExternalInput")
    d_b2r = nc.dram_tensor("b2r", [128, 2, NB, BPC], F32, kind="ExternalInput")
    d_id = nc.dram_tensor("ident", [128, 128], F32, kind="ExternalInput")
    d_out = nc.dram_tensor("out", [128, BPC, 2, HW], BF16, kind="ExternalOutput")

    with tile.TileContext(nc) as tc, ExitStack() as ctx:
        prm = ctx.enter_context(tc.tile_pool(name="prm", bufs=1))
        xp = ctx.enter_context(tc.tile_pool(name="xp", bufs=3))
        z24p = ctx.enter_context(tc.tile_pool(name="z24p", bufs=2))
        zsp = ctx.enter_context(tc.tile_pool(name="zsp", bufs=6))
        op = ctx.enter_context(tc.tile_pool(name="op", bufs=3))
        mtp = ctx.enter_context(tc.tile_pool(name="mtp", bufs=2))
        sm = ctx.enter_context(tc.tile_pool(name="sm", bufs=2))
        ps_c = ctx.enter_context(tc.tile_pool(name="ps_c", bufs=3, space="PSUM"))
        ps_m = ctx.enter_context(tc.tile_pool(name="ps_m", bufs=2, space="PSUM"))
        ps_s = ctx.enter_context(tc.tile_pool(name="ps_s", bufs=1, space="PSUM"))
        ps_u = ctx.enter_context(tc.tile_pool(name="ps_u", bufs=1, space="PSUM"))

        # ---- params (loaded once, sync HW ring) ----
        ft_sb = prm.tile([128, 2, 3, 128], BF16, tag="ft")
        nc.sync.dma_start(out=ft_sb, in_=d_ft[:, :, :, :])
        w1tb_sb = prm.tile([128, 2, HID], BF16, tag="w1tb")
        nc.sync.dma_start(out=w1tb_sb, in_=d_w1tb[:, :, :])
        b1_sb = prm.tile([HID, 1], F32, tag="b1")
        nc.sync.dma_start(out=b1_sb, in_=d_b1[:, :])
        w2p_sb = prm.tile([HID, 2, NB, 128], F32, tag="w2p")
        nc.sync.dma_start(out=w2p_sb, in_=d_w2p[:, :, :, :])
        b2r_sb = prm.tile([128, 2, NB, BPC], F32, tag="b2r")
        nc.sync.dma_start(out=b2r_sb, in_=d_b2r[:, :, :, :])
        id_sb = prm.tile([128, 128], F32, tag="ident")
        nc.sync.dma_start(out=id_sb, in_=d_id[:, :])

        hraw_sb = prm.tile([HID, BPC], F32, tag="hraw")
        mr_sb = prm.tile([128, M72], F32, tag="mr")
        nc.vector.memset(mr_sb, 0)   # gap columns stay zero forever
        h_sb = prm.tile([HID, BPC], F32, tag="h")

        xts = [None] * BPC
        zss = [None] * BPC
        mixTs = [None] * 2   # per batch of 4
        recs = [None] * 2

        def mlp_batch(b):
            j0 = 4 * b
            nc.scalar.activation(out=h_sb[:, j0:j0 + 4],
                                 in_=hraw_sb[:, j0:j0 + 4], func=AFT.Relu,
                                 bias=b1_sb, scale=1.0)
            pl = ps_s.tile([128, 2, NB, 4], F32, tag="pl")
            for oc in range(2):
                for n in range(NB):
                    nc.tensor.matmul(pl[:, oc, n, :], w2p_sb[:, oc, n, :],
                                     h_sb[:, j0:j0 + 4], start=True, stop=True)
            lg = sm.tile([128, 2, NB, 4], F32, tag="lg")
            nc.vector.tensor_tensor(out=lg, in0=pl, in1=b2r_sb[:, :, :, 0:4],
                                    op=ALU.add)
            ex = sm.tile([128, 2, NB, 4], F32, tag="ex")
            nc.scalar.activation(out=ex, in_=lg, func=AFT.Exp)
            # transpose (n, j) -> (j, n) so the n-sum is innermost
            exT = sm.tile([128, 2, 4, NB], F32, tag="exT")
            nc.vector.tensor_copy(exT, ex.rearrange("p a n j -> p a j n"))
            sums = sm.tile([128, 2, 4], F32, tag="sums")
            nc.vector.reduce_sum(sums, exT, axis=AXX)
            rec_b = mtp.tile([128, 2, 4], F32, tag="rec")
            nc.vector.reciprocal(rec_b, sums)
            recs[b] = rec_b
            mixT_b = mtp.tile([M72, 2, 4, 128], BF16, tag="mixT")
            mixTs[b] = mixT_b
            for j2 in range(4):
                for oc in range(2):
                    for dx in range(3):
                        nc.vector.tensor_copy(mr_sb[:, 32 * dx:32 * dx + 8],
                                              exT[:, oc, j2, :])
                    ptr = ps_s.tile([M72, 128], F32, tag="ptr")
                    nc.tensor.transpose(ptr, mr_sb, id_sb)
                    nc.vector.tensor_copy(mixT_b[:, oc, j2, :], ptr)

        for j in range(BPC):
            # ---- load x (SWDGE ring; first sample on sync HW ring) ----
            xt = xp.tile([128, 2, HW], BF16, tag="x")
            xts[j] = xt
            nc.gpsimd.dma_start(out=xt, in_=d_x[:, j, :, :])

            # ---- conv: accumulate 3 dy taps x 2 channel halves in PSUM ----
            z24 = z24p.tile([M72, ZLEN], BF16, tag="z24")
            if j < 2:
                nc.vector.memset(z24[:, 0:1], 0)
                nc.vector.memset(z24[:, ZLEN - 1:ZLEN], 0)
            ups = ps_u.tile([HID, 512], F32, tag="u")
            for hc in range(8):
                yps = ps_c.tile([128, 512], F32, tag="yps")
                dys = (1, 2, 0) if hc == 7 else (1, 0, 2)
                k = 0
                for dy in dys:
                    lo = 512 * hc + 64 * (dy - 1)
                    hi = lo + 512
                    clo, chi = max(lo, 0), min(hi, HW)
                    for half in range(2):
                        nc.tensor.matmul(
                            yps[:, clo - lo:512 - (hi - chi)],
                            ft_sb[:, half, dy, :],
                            xt[:, half, clo:chi],
                            start=(k == 0), stop=(k == 5))
                        k += 1
                for half in range(2):
                    nc.tensor.matmul(
                        ups, w1tb_sb[:, half, :],
                        xt[:, half, 512 * hc:512 * (hc + 1)],
                        start=(hc == 0 and half == 0),
                        stop=(hc == 7 and half == 1))
                nc.scalar.copy(
                    out=z24[:, 1 + 512 * hc:1 + 512 * (hc + 1)], in_=yps[0:72, :])
            nc.vector.reduce_sum(hraw_sb[:, j:j + 1], ups, axis=AXX)

            # ---- dx shift: z24 -> zs on DVE/GpSimd, then wrap fixups ----
            zs = zsp.tile([M72, HW], BF16, tag="zs")
            zss[j] = zs
            if j < 6:
                # first use of each pool buffer: zero it so gap rows and
                # wrap columns are zero forever after
                nc.vector.memset(zs[0:64, :], 0)
                nc.vector.memset(zs[64:72, :], 0)
            nc.sync.dma_start(out=zs[0:8, :], in_=z24[0:8, 0:HW])
            nc.sync.dma_start(out=zs[32:40, :], in_=z24[32:40, 1:1 + HW])
            nc.sync.dma_start(out=zs[64:72, :], in_=z24[64:72, 2:2 + HW])
            zv = zs.rearrange("p (h w) -> p h w", w=64)
            nc.vector.memset(zv[0:8, :, 0:1], 0)
            nc.vector.memset(zv[64:72, :, 63:64], 0)

            # ---- mix deferred so conv matmuls fill the MLP latency ----
            if j == 4:
                mlp_batch(0)
            if j == 7:
                mlp_batch(1)
            if j == 5 or j == 7:
                b = (j - 5) // 2
                mixT_b = mixTs[b]
                rec_b = recs[b]
                for j2 in range(4):
                    jj = 4 * b + j2
                    zsj = zss[jj]
                    ot = op.tile([128, 2, HW], BF16, tag="out")
                    for oc in range(2):
                        for hc in range(8):
                            om = ps_m.tile([128, 512], F32, tag="om")
                            nc.tensor.matmul(om, mixT_b[:, oc, j2, :],
                                             zsj[:, 512 * hc:512 * (hc + 1)],
                                             start=True, stop=True)
                            dst = ot[:, oc, 512 * hc:512 * (hc + 1)]
                            if hc % 2 == 0:
                                nc.vector.tensor_scalar_mul(
                                    out=dst, in0=om,
                                    scalar1=rec_b[:, oc, j2:j2 + 1])
                            else:
                                nc.scalar.activation(
                                    out=dst, in_=om, func=AFT.Copy,
                                    scale=rec_b[:, oc, j2:j2 + 1])
                    oring = nc.gpsimd if jj == 0 else nc.scalar
                    oring.dma_start(out=d_out[:, jj, :, :], in_=ot)

    nc.compile()
    return nc


def _prep_inputs(x, w1, b1, w2, b2, base_filters):
    """Host-side input layout prep. Returns per-core in_maps."""
    B = x.shape[0]
    # x[core*8+j, 2p+half, hw] -> [128(p), 8(j), 2(half), hw] bf16
    xs = x.reshape(B, 128, 2, HW)
    # w1t[p, half, hid] = w1[hid, 2p+half] / HW  (mean folded in)
    w1t = (np.ascontiguousarray(
        w1.reshape(HID, 128, 2).transpose(1, 2, 0)).astype(np.float32)
        / float(HW)).astype(ml_dtypes.bfloat16)
    b1c = np.ascontiguousarray(b1.reshape(HID, 1)).astype(np.float32)
    # w2 row index = o*NB + n with o = 2p+oc -> w2p[hid, oc, n, p]
    w2r = w2.reshape(128, 2, NB, HID)          # [p, oc, n, hid]
    w2p = np.ascontiguousarray(w2r.transpose(3, 1, 2, 0)).astype(np.float32)
    b2r = np.broadcast_to(
        b2.reshape(128, 2, NB, 1), (128, 2, NB, BPC))
    b2r = np.ascontiguousarray(b2r).astype(np.float32)
    # ft[p, half, dy, 32dx+n] = filt[n, 2p+half, dy, dx]; gaps zero
    filt = base_filters.reshape(NB, 128, 2, 3, 3)
    fpv = filt.transpose(1, 2, 3, 4, 0)                 # [p, half, dy, dx, n]
    ft = np.zeros((128, 2, 3, 128), dtype=np.float32)
    for dx in range(3):
        ft[:, :, :, 32 * dx:32 * dx + NB] = fpv[:, :, :, dx, :]
    ft = ft.astype(ml_dtypes.bfloat16)
    ident = np.eye(128, dtype=np.float32)

    in_maps = []
    for core in range(N_CORES):
        xc = np.ascontiguousarray(
            xs[core * BPC:(core + 1) * BPC].transpose(1, 0, 2, 3)).astype(
                ml_dtypes.bfloat16)
        in_maps.append({
            "x": xc, "ft": ft, "w1tb": w1t, "b1": b1c, "w2p": w2p,
            "b2r": b2r, "ident": ident,
        })
    return in_maps


def kernel(x, w1, b1, w2, b2, base_filters):
    global _BUILT
    if _BUILT is None:
        _BUILT = _build()
    nc = _BUILT
    in_maps = _prep_inputs(np.asarray(x, dtype=np.float32),
                           np.asarray(w1, dtype=np.float32),
                           np.asarray(b1, dtype=np.float32),
                           np.asarray(w2, dtype=np.float32),
                           np.asarray(b2, dtype=np.float32),
                           np.asarray(base_filters, dtype=np.float32))
    res = run_bass_kernel_spmd(nc, in_maps, core_ids=list(range(N_CORES)))
    outs = []
    for core in range(N_CORES):
        o = res.results[core]["out"]            # [128, BPC, 2, HW] bf16
        o = np.asarray(o).astype(np.float32).transpose(1, 0, 2, 3)
        outs.append(o.reshape(BPC, CO, H, W))
    return np.concatenate(outs, axis=0).astype(np.float32)
